# revision 2
# baseline (speedup 1.0000x reference)
"""Trainium2 Bass kernel for DigitConvolutionalModel.

Math: the 3x3 valid conv is a linear map, so it folds into the first Linear
layer on the host (O(1) w.r.t. batch):  out = relu(x @ W_eff + b1) @ w2.T + b2
with W_eff[784, 128].  Distribution is pure data parallel: batch sharded
across 8 NeuronCores, weights replicated, each core computing [10, 8192].

dtypes: x ships as fp8 e3m4 (4 mantissa bits; subnormals handled exactly by
the PE's FP22 upconvert) against fp16 weights — this halves HBM traffic vs
fp16 and costs ~1.3e-2 max rel error on this data (gate: 2e-2).  fp32 PSUM
accumulation; h is emitted fp16 for the second matmul; logits are stored
fp16 (4.9e-4 relative — noise here) and upconverted on host.

Two PE-array packing tricks remove the under-utilized matmul passes:

- The K=16 remainder matmul wastes 112/128 PE rows.  Batch tiles group
  in quads: tile 4q+j's remainder runs in PE row-strip 32j via
  tile_position=(32j, 0).  Row-disjoint matmuls execute concurrently
  (Dstart ~4ns), so 4 remainder passes cost ~1 pass.
- The M=10 second matmul wastes 118/128 PE columns.  Per quad, the 4 mm2s
  run in column strips via tile_position=(0, 32j), writing partition strip
  32j..32j+9 of one shared PSUM bank.  4 passes cost ~1.

Per quad: 24 full mm1 passes + 1 remainder burst + 1 mm2 burst = 26 passes
vs 32 unpacked.

DMA plan (TRN2 has exactly two HWDGE rings — sync + scalar — plus the
gpsimd SWDGE): x streams on both HWDGE rings in 2-tile groups so each
descriptor covers a 6144B per-partition line (double the baseline's 3072B,
halving the ring's descriptor-generation work per byte).  wm and batch
tiles 0/1 are split k-wise across the two rings so the first mm1 chain's
k=0..2 matmuls can start after ~300KB instead of ~600KB.  Everything small
(w2/b/wr4/xr4) plus all four output stores ride the otherwise-idle gpsimd
SWDGE queue, keeping the HWDGE rings 100% on x.

The quad epilogue pipeline: quad q's mm2 burst is emitted after quad q+1's
mm1 chains, so the PE never waits on the relu chain.  The four relus per
quad are split two-ways (vector tensor_scalar / scalar activation) so the
chain is 2 deep per engine instead of 4 on vector alone; b2 is replicated
per row-strip so one op biases all 4 tiles; each quad stores its whole
[128, 512] fp16 strip block in one SWDGE DMA (rows 10..31 of each strip
are don't-care) and the host extracts the 10 valid rows per strip.
"""

import numpy as np
import ml_dtypes

import concourse.bass as bass  # noqa: F401  (bass registers mybir lowerings)
import concourse.mybir as mybir
import concourse.tile as tile
from concourse import bacc
from concourse.bass_utils import run_bass_kernel_spmd

N_CORES = 8
B = 65536
B_SH = B // N_CORES  # 8192 rows per core
D = 784              # 28*28 input features
DM = 768             # features in the main 128-partition stream
DR = D - DM          # 16 remainder features
H = 128              # hidden
OUT = 10
KT = 128             # contraction tile = full partition dim
NK = DM // KT        # 6 main K-tiles
NB = 512             # batch columns per tile (= one fp32 PSUM bank)
NT = B_SH // NB      # 16 batch tiles
NQ = NT // 4         # quads of batch tiles

N_WARM = 6   # bridge PE from engine-start (~7.4us) to t0 arrival with no
             # idle gap, so the HAM clock never re-throttles

_CACHE = {}


def _build_nc():
    f32 = mybir.dt.float32
    f16 = mybir.dt.float16
    f8 = mybir.dt.float8e3
    nc = bacc.Bacc("TRN2", target_bir_lowering=False, debug=False,
                   num_devices=N_CORES)
    xtp = nc.dram_tensor("xtp", [KT, NT, NK, NB], f8,
                         kind="ExternalInput").ap()
    # remainder features per row-strip: [32j+r, q, c] = feature 768+r of
    # batch tile 4q+j (r<16; rows 16..31 of each strip are zero padding)
    xr4 = nc.dram_tensor("xr4", [KT, NQ, NB], f8, kind="ExternalInput").ap()
    wm = nc.dram_tensor("wm", [KT, NK, H], f16, kind="ExternalInput").ap()
    # remainder weights replicated into each row-strip
    wr4 = nc.dram_tensor("wr4", [KT, H], f16, kind="ExternalInput").ap()
    w2t = nc.dram_tensor("w2t", [H, OUT], f16, kind="ExternalInput").ap()
    # biasd[:, 0] = b1; biasd[32j+r, 1] = b2[r] (r<10)
    biasd = nc.dram_tensor("biasd", [KT, 2], f32, kind="ExternalInput").ap()
    # out4[32j+r, q, c] = logit r of batch row (4q+j)*512+c (r<10; rows
    # 10..31 of each strip are don't-care) — one store per quad keeps the
    # tail short (each store trigger costs ~0.65us of engine time)
    out4 = nc.dram_tensor("out4", [KT, NQ, NB], f16,
                          kind="ExternalOutput").ap()

    relu = mybir.ActivationFunctionType.Relu
    ident = mybir.ActivationFunctionType.Identity

    with tile.TileContext(nc) as tc:
        with (
            tc.tile_pool(name="wpool", bufs=1) as wpool,
            tc.tile_pool(name="xpool", bufs=1) as xpool,
            tc.tile_pool(name="hpool", bufs=8) as hpool,
            tc.tile_pool(name="opool", bufs=2) as opool,
            tc.tile_pool(name="ps1", bufs=4, space="PSUM") as ps1pool,
            tc.tile_pool(name="ps2", bufs=2, space="PSUM") as ps2pool,
        ):
            x_sb = xpool.tile([KT, NT, NK, NB], f8)
            w_sb = wpool.tile([KT, NK, H], f16)
            w2_sb = wpool.tile([H, OUT], f16)
            bias_sb = wpool.tile([KT, 2], f32)
            wr_sb = wpool.tile([KT, H], f16)
            xr_sb = wpool.tile([KT, NQ, NB], f8)

            # k-split leads: each ring carries half of wm then half of
            # tiles 0/1, so the first chain's k=0..2 (sync half) and
            # k=3..5 (scalar half) both unblock after ~300KB per ring.
            nc.sync.dma_start(w_sb[:, 0:3, :], wm[:, 0:3, :])
            nc.scalar.dma_start(w_sb[:, 3:6, :], wm[:, 3:6, :])
            for t in range(2):
                nc.sync.dma_start(x_sb[:, t:t + 1, 0:3, :],
                                  xtp[:, t:t + 1, 0:3, :])
                nc.scalar.dma_start(x_sb[:, t:t + 1, 3:6, :],
                                    xtp[:, t:t + 1, 3:6, :])
            # small tensors on the SWDGE queue (idle until the stores)
            nc.gpsimd.dma_start(bias_sb[:], biasd[:])
            nc.gpsimd.dma_start(wr_sb[:], wr4[:])
            nc.gpsimd.dma_start(w2_sb[:], w2t[:])
            nc.gpsimd.dma_start(xr_sb[:], xr4[:])
            # 2-tile groups, alternating rings in consumption order
            for gi, t in enumerate(range(2, NT, 2)):
                eng = (nc.sync, nc.scalar)[gi % 2]
                eng.dma_start(x_sb[:, t:t + 2, :, :], xtp[:, t:t + 2, :, :])

            warm_x = wpool.tile([KT, NB], f16)
            nc.vector.memset(warm_x[:], 0.0)
            warm_ps = ps1pool.tile([H, NB], f32, tag="ps1")
            for _ in range(N_WARM):
                nc.tensor.matmul(warm_ps[:], lhsT=warm_x[:, 0:H],
                                 rhs=warm_x[:], start=True, stop=True)

            def mm2_store_burst(q, hs):
                # 4 col-tiled mm2 passes into one shared PSUM bank
                ps2 = ps2pool.tile([KT, NB], f32, name="ps2")
                for j in range(4):
                    nc.tensor.matmul(
                        ps2[32 * j:32 * j + OUT, :],
                        lhsT=w2_sb[:], rhs=hs[j][:],
                        start=True, stop=True,
                        tile_position=(0, 32 * j),
                    )
                o_sb = opool.tile([KT, NB], f16, name="o_sb")
                if q % 2 == 0:
                    nc.vector.tensor_scalar_add(o_sb[:], ps2[:],
                                                bias_sb[:, 1:2])
                else:
                    nc.scalar.activation(o_sb[:], ps2[:], ident,
                                         bias=bias_sb[:, 1:2])
                nc.gpsimd.dma_start(out4[:, q, :], o_sb[:])

            prev = None
            for q in range(NQ):
                ps1s = []
                for j in range(4):
                    t = 4 * q + j
                    ps1 = ps1pool.tile([H, NB], f32, name="ps1")
                    for k in range(NK):
                        nc.tensor.matmul(
                            ps1[:],
                            lhsT=w_sb[:, k, :],
                            rhs=x_sb[:, t, k, :],
                            start=(k == 0),
                            stop=False,
                        )
                    ps1s.append(ps1)
                # remainder burst: 4 row-tiled K=16 passes, one per strip
                for j in range(4):
                    nc.tensor.matmul(
                        ps1s[j][:],
                        lhsT=wr_sb[32 * j:32 * j + DR, :],
                        rhs=xr_sb[32 * j:32 * j + DR, q, :],
                        start=False, stop=True,
                        tile_position=(32 * j, 0),
                    )
                # relu split across DVE and Act so the chain is 2 deep
                # per engine; emission order lets both start immediately
                hs = [None] * 4
                for j in (0, 2, 1, 3):
                    h_sb = hpool.tile([H, NB], f16, name="h_sb")
                    if j < 2:
                        nc.vector.tensor_scalar(
                            h_sb[:], ps1s[j][:], bias_sb[:, 0:1], 0.0,
                            mybir.AluOpType.add, mybir.AluOpType.max)
                    else:
                        nc.scalar.activation(h_sb[:], ps1s[j][:], relu,
                                             bias=bias_sb[:, 0:1])
                    hs[j] = h_sb
                if prev is not None:
                    mm2_store_burst(*prev)
                prev = (q, hs)
            mm2_store_burst(*prev)

    nc.compile()
    return nc


def _get_nc():
    if "nc" not in _CACHE:
        _CACHE["nc"] = _build_nc()
    return _CACHE["nc"]


def _fold_weights(conv_w: np.ndarray, w1: np.ndarray) -> np.ndarray:
    """W_eff[784, 128]: h_pre = x @ W_eff  ==  conv(x) @ w1.T  (float64 accum)."""
    w1k = w1.reshape(H, 26, 26).transpose(1, 2, 0).astype(np.float64)  # [i,j,k]
    cw = conv_w.astype(np.float64)
    W = np.zeros((28, 28, H), np.float64)
    for di in range(3):
        for dj in range(3):
            W[di:di + 26, dj:dj + 26, :] += cw[di, dj] * w1k
    return W.reshape(D, H).astype(np.float32)


def make_in_maps(x, conv_w, w1, b1, w2, b2):
    x = np.asarray(x, np.float32)
    weff = _fold_weights(np.asarray(conv_w, np.float32),
                         np.asarray(w1, np.float32))
    wm = np.ascontiguousarray(
        weff[:DM].reshape(NK, KT, H).transpose(1, 0, 2)).astype(np.float16)
    wr4 = np.zeros((KT, H), np.float16)
    for j in range(4):
        wr4[32 * j:32 * j + DR] = weff[DM:].astype(np.float16)
    w2t = np.ascontiguousarray(np.asarray(w2, np.float32).T).astype(np.float16)
    biasd = np.zeros((KT, 2), np.float32)
    biasd[:, 0] = np.asarray(b1, np.float32)
    for j in range(4):
        biasd[32 * j:32 * j + OUT, 1] = np.asarray(b2, np.float32)
    in_maps = []
    for i in range(N_CORES):
        xq = x[i * B_SH:(i + 1) * B_SH].astype(ml_dtypes.float8_e3m4)
        xtp = xq[:, :DM].reshape(NT, NB, NK, KT).transpose(3, 0, 2, 1)
        # [q, j, c, r] -> strip layout [4j, r, q, c] padded to 32 rows/strip
        r16 = xq[:, DM:].reshape(NQ, 4, NB, DR).transpose(1, 3, 0, 2)
        xr4 = np.zeros((4, 32, NQ, NB), ml_dtypes.float8_e3m4)
        xr4[:, :DR] = r16
        in_maps.append({"xtp": np.ascontiguousarray(xtp),
                        "xr4": np.ascontiguousarray(xr4.reshape(KT, NQ, NB)),
                        "wm": wm, "wr4": wr4, "w2t": w2t, "biasd": biasd})
    return in_maps


def kernel(x, conv_w, w1, b1, w2, b2):
    nc = _get_nc()
    in_maps = make_in_maps(x, conv_w, w1, b1, w2, b2)
    res = run_bass_kernel_spmd(nc, in_maps, list(range(N_CORES)))
    # out4[32j+r, q, c] -> out[(4q+j)*512+c, r]
    outs = []
    for i in range(N_CORES):
        o4 = res.results[i]["out4"].astype(np.float32)
        o4 = o4.reshape(4, 32, NQ, NB)[:, :OUT]
        outs.append(o4.transpose(1, 2, 0, 3).reshape(OUT, B_SH))
    out = np.concatenate(outs, axis=1)
    return np.ascontiguousarray(out.T)  # [65536, 10] float32


# revision 5
# speedup vs baseline: 1.0077x; 1.0077x over previous
"""Trainium2 Bass kernel for DigitConvolutionalModel.

Math: the 3x3 valid conv is a linear map, so it folds into the first Linear
layer on the host (O(1) w.r.t. batch):  out = relu(x @ W_eff + b1) @ w2.T + b2
with W_eff[784, 128].  Distribution is pure data parallel: batch sharded
across 8 NeuronCores, weights replicated, each core computing [10, 8192].

dtypes: x ships as fp8 e3m4 (4 mantissa bits; subnormals handled exactly by
the PE's FP22 upconvert) against fp16 weights — this halves HBM traffic vs
fp16 and costs ~1.3e-2 max rel error on this data (gate: 2e-2).  fp32 PSUM
accumulation; h is emitted fp16 for the second matmul; logits are stored
fp16 (4.9e-4 relative — noise here) and upconverted on host.

Two PE-array packing tricks remove the under-utilized matmul passes:

- The K=16 remainder matmul wastes 112/128 PE rows.  Batch tiles group
  in quads: tile 4q+j's remainder runs in PE row-strip 32j via
  tile_position=(32j, 0).  Row-disjoint matmuls execute concurrently
  (Dstart ~4ns), so 4 remainder passes cost ~1 pass.
- The M=10 second matmul wastes 118/128 PE columns.  Per quad, the 4 mm2s
  run in column strips via tile_position=(0, 32j), writing partition strip
  32j..32j+9 of one shared PSUM bank.  4 passes cost ~1.

Per quad: 24 full mm1 passes + 1 remainder burst + 1 mm2 burst = 26 passes
vs 32 unpacked.

DMA plan (TRN2 has exactly two HWDGE rings — sync + scalar — plus the
gpsimd SWDGE): x streams on both HWDGE rings in 2-tile groups so each
descriptor covers a 6144B per-partition line (double the baseline's 3072B,
halving the ring's descriptor-generation work per byte).  wm and batch
tiles 0/1 are split k-wise across the two rings so the first mm1 chain's
k=0..2 matmuls can start after ~300KB instead of ~600KB.  Everything small
(w2/b/wr4/xr4) plus all four output stores ride the otherwise-idle gpsimd
SWDGE queue, keeping the HWDGE rings 100% on x.

The quad epilogue pipeline: quad q's mm2 burst is emitted after quad q+1's
mm1 chains, so the PE never waits on the relu chain.  The four relus per
quad are split two-ways (vector tensor_scalar / scalar activation) so the
chain is 2 deep per engine instead of 4 on vector alone; b2 is replicated
per row-strip so one op biases all 4 tiles; each quad stores its whole
[128, 512] fp16 strip block in one SWDGE DMA (rows 10..31 of each strip
are don't-care) and the host extracts the 10 valid rows per strip.
"""

import numpy as np
import ml_dtypes

import concourse.bass as bass  # noqa: F401  (bass registers mybir lowerings)
import concourse.mybir as mybir
import concourse.tile as tile
from concourse import bacc
from concourse.bass_utils import run_bass_kernel_spmd

N_CORES = 8
B = 65536
B_SH = B // N_CORES  # 8192 rows per core
D = 784              # 28*28 input features
DM = 768             # features in the main 128-partition stream
DR = D - DM          # 16 remainder features
H = 128              # hidden
OUT = 10
KT = 128             # contraction tile = full partition dim
NK = DM // KT        # 6 main K-tiles
NB = 512             # batch columns per tile (= one fp32 PSUM bank)
NT = B_SH // NB      # 16 batch tiles
NQ = NT // 4         # quads of batch tiles

N_WARM = 8   # bridge PE from engine-start (~7.4us) to t0 arrival with no
             # idle gap, so the HAM clock never re-throttles

_CACHE = {}


def _build_nc():
    f32 = mybir.dt.float32
    f16 = mybir.dt.float16
    f8 = mybir.dt.float8e3
    nc = bacc.Bacc("TRN2", target_bir_lowering=False, debug=False,
                   num_devices=N_CORES)
    xtp = nc.dram_tensor("xtp", [KT, NT, NK, NB], f8,
                         kind="ExternalInput").ap()
    # remainder features per row-strip: [32j+r, q, c] = feature 768+r of
    # batch tile 4q+j (r<16; rows 16..31 of each strip are zero padding)
    xr4 = nc.dram_tensor("xr4", [KT, NQ, NB], f8, kind="ExternalInput").ap()
    wm = nc.dram_tensor("wm", [KT, NK, H], f16, kind="ExternalInput").ap()
    # remainder weights replicated into each row-strip
    wr4 = nc.dram_tensor("wr4", [KT, H], f16, kind="ExternalInput").ap()
    w2t = nc.dram_tensor("w2t", [H, OUT], f16, kind="ExternalInput").ap()
    # biasd[:, 0] = b1; biasd[32j+r, 1] = b2[r] (r<10)
    biasd = nc.dram_tensor("biasd", [KT, 2], f32, kind="ExternalInput").ap()
    # out4[32j+r, q, c] = logit r of batch row (4q+j)*512+c (r<10; rows
    # 10..31 of each strip are don't-care) — one store per quad keeps the
    # tail short (each store trigger costs ~0.65us of engine time)
    out4 = nc.dram_tensor("out4", [KT, NQ, NB], f16,
                          kind="ExternalOutput").ap()

    relu = mybir.ActivationFunctionType.Relu
    ident = mybir.ActivationFunctionType.Identity

    with tile.TileContext(nc) as tc:
        with (
            tc.tile_pool(name="wpool", bufs=1) as wpool,
            tc.tile_pool(name="xpool", bufs=1) as xpool,
            tc.tile_pool(name="hpool", bufs=8) as hpool,
            tc.tile_pool(name="opool", bufs=2) as opool,
            tc.tile_pool(name="ps1", bufs=4, space="PSUM") as ps1pool,
            tc.tile_pool(name="ps2", bufs=2, space="PSUM") as ps2pool,
        ):
            x_sb = xpool.tile([KT, NT, NK, NB], f8)
            w_sb = wpool.tile([KT, NK, H], f16)
            w2_sb = wpool.tile([H, OUT], f16)
            bias_sb = wpool.tile([KT, 2], f32)
            wr_sb = wpool.tile([KT, H], f16)
            xr_sb = wpool.tile([KT, NQ, NB], f8)

            # The HWDGE ring cost is ~21ns per descriptor per queue, and
            # every SBUF-bound DMA needs one descriptor per partition —
            # so a 2-tile group (6144B lines) delivers two tiles in the
            # same ~2.7us one tile would take.  All-pairs, alternating
            # rings in consumption order: every tile lands well before
            # the PE needs it and the rings run ~200GB/s each.
            for gi, t in enumerate(range(0, NT, 2)):
                eng = (nc.sync, nc.scalar)[gi % 2]
                eng.dma_start(x_sb[:, t:t + 2, :, :], xtp[:, t:t + 2, :, :])
            # wm + small tensors ride the SWDGE queue (software descriptor
            # gen, runs concurrently with both rings); wm lands ~10.5us,
            # just before the first pair (~10.8us) unblocks the chain.
            nc.gpsimd.dma_start(w_sb[:], wm[:])
            nc.gpsimd.dma_start(bias_sb[:], biasd[:])
            nc.gpsimd.dma_start(wr_sb[:], wr4[:])
            nc.gpsimd.dma_start(w2_sb[:], w2t[:])
            nc.gpsimd.dma_start(xr_sb[:], xr4[:])

            warm_x = wpool.tile([KT, NB], f16)
            nc.vector.memset(warm_x[:], 0.0)
            warm_ps = ps1pool.tile([H, NB], f32, tag="ps1")
            for _ in range(N_WARM):
                nc.tensor.matmul(warm_ps[:], lhsT=warm_x[:, 0:H],
                                 rhs=warm_x[:], start=True, stop=True)

            def mm2_store_burst(q, hs):
                # 4 col-tiled mm2 passes into one shared PSUM bank
                ps2 = ps2pool.tile([KT, NB], f32, name="ps2")
                for j in range(4):
                    nc.tensor.matmul(
                        ps2[32 * j:32 * j + OUT, :],
                        lhsT=w2_sb[:], rhs=hs[j][:],
                        start=True, stop=True,
                        tile_position=(0, 32 * j),
                    )
                # b2-add in partition halves on both engines so each half
                # can store as soon as it lands
                o_sb = opool.tile([KT, NB], f16, name="o_sb")
                nc.vector.tensor_scalar_add(o_sb[0:64, :], ps2[0:64, :],
                                            bias_sb[0:64, 1:2])
                nc.scalar.activation(o_sb[64:KT, :], ps2[64:KT, :], ident,
                                     bias=bias_sb[64:KT, 1:2])
                if q == NQ - 1:
                    # final store on the (by now idle) HWDGE rings: each
                    # half fires the moment its bias-add completes
                    nc.sync.dma_start(out4[0:64, q, :], o_sb[0:64, :])
                    nc.scalar.dma_start(out4[64:KT, q, :], o_sb[64:KT, :])
                else:
                    nc.gpsimd.dma_start(out4[:, q, :], o_sb[:])

            prev = None
            for q in range(NQ):
                ps1s = []
                for j in range(4):
                    t = 4 * q + j
                    ps1 = ps1pool.tile([H, NB], f32, name="ps1")
                    for k in range(NK):
                        nc.tensor.matmul(
                            ps1[:],
                            lhsT=w_sb[:, k, :],
                            rhs=x_sb[:, t, k, :],
                            start=(k == 0),
                            stop=False,
                        )
                    ps1s.append(ps1)
                # remainder burst: 4 row-tiled K=16 passes, one per strip
                for j in range(4):
                    nc.tensor.matmul(
                        ps1s[j][:],
                        lhsT=wr_sb[32 * j:32 * j + DR, :],
                        rhs=xr_sb[32 * j:32 * j + DR, q, :],
                        start=False, stop=True,
                        tile_position=(32 * j, 0),
                    )
                # relu split across DVE and Act so the chain is 2 deep
                # per engine; emission order lets both start immediately
                hs = [None] * 4
                for j in (0, 2, 1, 3):
                    h_sb = hpool.tile([H, NB], f16, name="h_sb")
                    if j < 2:
                        nc.vector.tensor_scalar(
                            h_sb[:], ps1s[j][:], bias_sb[:, 0:1], 0.0,
                            mybir.AluOpType.add, mybir.AluOpType.max)
                    else:
                        nc.scalar.activation(h_sb[:], ps1s[j][:], relu,
                                             bias=bias_sb[:, 0:1])
                    hs[j] = h_sb
                if prev is not None:
                    mm2_store_burst(*prev)
                prev = (q, hs)
            mm2_store_burst(*prev)

    nc.compile()
    return nc


def _get_nc():
    if "nc" not in _CACHE:
        _CACHE["nc"] = _build_nc()
    return _CACHE["nc"]


def _fold_weights(conv_w: np.ndarray, w1: np.ndarray) -> np.ndarray:
    """W_eff[784, 128]: h_pre = x @ W_eff  ==  conv(x) @ w1.T  (float64 accum)."""
    w1k = w1.reshape(H, 26, 26).transpose(1, 2, 0).astype(np.float64)  # [i,j,k]
    cw = conv_w.astype(np.float64)
    W = np.zeros((28, 28, H), np.float64)
    for di in range(3):
        for dj in range(3):
            W[di:di + 26, dj:dj + 26, :] += cw[di, dj] * w1k
    return W.reshape(D, H).astype(np.float32)


def make_in_maps(x, conv_w, w1, b1, w2, b2):
    x = np.asarray(x, np.float32)
    weff = _fold_weights(np.asarray(conv_w, np.float32),
                         np.asarray(w1, np.float32))
    wm = np.ascontiguousarray(
        weff[:DM].reshape(NK, KT, H).transpose(1, 0, 2)).astype(np.float16)
    wr4 = np.zeros((KT, H), np.float16)
    for j in range(4):
        wr4[32 * j:32 * j + DR] = weff[DM:].astype(np.float16)
    w2t = np.ascontiguousarray(np.asarray(w2, np.float32).T).astype(np.float16)
    biasd = np.zeros((KT, 2), np.float32)
    biasd[:, 0] = np.asarray(b1, np.float32)
    for j in range(4):
        biasd[32 * j:32 * j + OUT, 1] = np.asarray(b2, np.float32)
    in_maps = []
    for i in range(N_CORES):
        xq = x[i * B_SH:(i + 1) * B_SH].astype(ml_dtypes.float8_e3m4)
        xtp = xq[:, :DM].reshape(NT, NB, NK, KT).transpose(3, 0, 2, 1)
        # [q, j, c, r] -> strip layout [4j, r, q, c] padded to 32 rows/strip
        r16 = xq[:, DM:].reshape(NQ, 4, NB, DR).transpose(1, 3, 0, 2)
        xr4 = np.zeros((4, 32, NQ, NB), ml_dtypes.float8_e3m4)
        xr4[:, :DR] = r16
        in_maps.append({"xtp": np.ascontiguousarray(xtp),
                        "xr4": np.ascontiguousarray(xr4.reshape(KT, NQ, NB)),
                        "wm": wm, "wr4": wr4, "w2t": w2t, "biasd": biasd})
    return in_maps


def kernel(x, conv_w, w1, b1, w2, b2):
    nc = _get_nc()
    in_maps = make_in_maps(x, conv_w, w1, b1, w2, b2)
    res = run_bass_kernel_spmd(nc, in_maps, list(range(N_CORES)))
    # out4[32j+r, q, c] -> out[(4q+j)*512+c, r]
    outs = []
    for i in range(N_CORES):
        o4 = res.results[i]["out4"].astype(np.float32)
        o4 = o4.reshape(4, 32, NQ, NB)[:, :OUT]
        outs.append(o4.transpose(1, 2, 0, 3).reshape(OUT, B_SH))
    out = np.concatenate(outs, axis=1)
    return np.ascontiguousarray(out.T)  # [65536, 10] float32


# revision 7
# speedup vs baseline: 1.0667x; 1.0585x over previous
"""Trainium2 Bass kernel for DigitConvolutionalModel.

Math: the 3x3 valid conv is a linear map, so it folds into the first Linear
layer on the host (O(1) w.r.t. batch):  out = relu(x @ W_eff + b1) @ w2.T + b2
with W_eff[784, 128].  Distribution is pure data parallel: batch sharded
across 8 NeuronCores, weights replicated, each core computing [10, 8192].

dtypes: x ships as fp8 e3m4 (4 mantissa bits; subnormals handled exactly by
the PE's FP22 upconvert) against fp16 weights — this halves HBM traffic vs
fp16 and costs ~1.3e-2 max rel error on this data (gate: 2e-2).  fp32 PSUM
accumulation; h is emitted fp16 for the second matmul; logits are stored
fp16 (4.9e-4 relative — noise here) and upconverted on host.

Two PE-array packing tricks remove the under-utilized matmul passes:

- The K=16 remainder matmul wastes 112/128 PE rows.  Batch tiles group
  in quads: tile 4q+j's remainder runs in PE row-strip 32j via
  tile_position=(32j, 0).  Row-disjoint matmuls execute concurrently
  (Dstart ~4ns), so 4 remainder passes cost ~1 pass.
- The M=10 second matmul wastes 118/128 PE columns.  Per quad, the 4 mm2s
  run in column strips via tile_position=(0, 32j), writing partition strip
  32j..32j+9 of one shared PSUM bank.  4 passes cost ~1.

Per quad: 24 full mm1 passes + 1 remainder burst + 1 mm2 burst = 26 passes
vs 32 unpacked.

DMA plan (TRN2 has exactly two HWDGE rings — sync + scalar — plus the
gpsimd SWDGE): x streams on both HWDGE rings in 2-tile groups so each
descriptor covers a 6144B per-partition line (double the baseline's 3072B,
halving the ring's descriptor-generation work per byte).  wm and batch
tiles 0/1 are split k-wise across the two rings so the first mm1 chain's
k=0..2 matmuls can start after ~300KB instead of ~600KB.  Everything small
(w2/b/wr4/xr4) plus all four output stores ride the otherwise-idle gpsimd
SWDGE queue, keeping the HWDGE rings 100% on x.

The quad epilogue pipeline: quad q's mm2 burst is emitted after quad q+1's
mm1 chains, so the PE never waits on the relu chain.  The four relus per
quad are split two-ways (vector tensor_scalar / scalar activation) so the
chain is 2 deep per engine instead of 4 on vector alone; b2 is replicated
per row-strip so one op biases all 4 tiles; each quad stores its whole
[128, 512] fp16 strip block in one SWDGE DMA (rows 10..31 of each strip
are don't-care) and the host extracts the 10 valid rows per strip.
"""

import numpy as np
import ml_dtypes

import concourse.bass as bass  # noqa: F401  (bass registers mybir lowerings)
import concourse.mybir as mybir
import concourse.tile as tile
from concourse import bacc
from concourse.bass_utils import run_bass_kernel_spmd

N_CORES = 8
B = 65536
B_SH = B // N_CORES  # 8192 rows per core
D = 784              # 28*28 input features
DM = 768             # features in the main 128-partition stream
DR = D - DM          # 16 remainder features
H = 128              # hidden
OUT = 10
KT = 128             # contraction tile = full partition dim
NK = DM // KT        # 6 main K-tiles
NB = 512             # batch columns per tile (= one fp32 PSUM bank)
NT = B_SH // NB      # 16 batch tiles
NQ = NT // 4         # quads of batch tiles

N_WARM = 14  # bridge PE from engine-start (~7.4us) to wm+pair(0,1) arrival
             # (~13.3us) with no idle gap, so the HAM clock never
             # re-throttles (a >1us PE gap halves the clock for ~6us)

_CACHE = {}


def _build_nc():
    f32 = mybir.dt.float32
    f16 = mybir.dt.float16
    f8 = mybir.dt.float8e3
    nc = bacc.Bacc("TRN2", target_bir_lowering=False, debug=False,
                   num_devices=N_CORES)
    xtp = nc.dram_tensor("xtp", [KT, NT, NK, NB], f8,
                         kind="ExternalInput").ap()
    # remainder features per row-strip: [32j+r, q, c] = feature 768+r of
    # batch tile 4q+j (r<16; rows 16..31 of each strip are zero padding)
    xr4 = nc.dram_tensor("xr4", [KT, NQ, NB], f8, kind="ExternalInput").ap()
    wm = nc.dram_tensor("wm", [KT, NK, H], f16, kind="ExternalInput").ap()
    # remainder weights replicated into each row-strip
    wr4 = nc.dram_tensor("wr4", [KT, H], f16, kind="ExternalInput").ap()
    w2t = nc.dram_tensor("w2t", [H, OUT], f16, kind="ExternalInput").ap()
    # biasd[:, 0] = b1; biasd[32j+r, 1] = b2[r] (r<10)
    biasd = nc.dram_tensor("biasd", [KT, 2], f32, kind="ExternalInput").ap()
    # out4[32j+r, q, c] = logit r of batch row (4q+j)*512+c (r<10; rows
    # 10..31 of each strip are don't-care) — one store per quad keeps the
    # tail short (each store trigger costs ~0.65us of engine time)
    out4 = nc.dram_tensor("out4", [KT, NQ, NB], f16,
                          kind="ExternalOutput").ap()

    relu = mybir.ActivationFunctionType.Relu
    ident = mybir.ActivationFunctionType.Identity

    with tile.TileContext(nc) as tc:
        with (
            tc.tile_pool(name="wpool", bufs=1) as wpool,
            tc.tile_pool(name="xpool", bufs=1) as xpool,
            tc.tile_pool(name="hpool", bufs=8) as hpool,
            tc.tile_pool(name="opool", bufs=2) as opool,
            tc.tile_pool(name="ps1", bufs=4, space="PSUM") as ps1pool,
            tc.tile_pool(name="ps2", bufs=2, space="PSUM") as ps2pool,
        ):
            x_sb = xpool.tile([KT, NT, NK, NB], f8)
            w_sb = wpool.tile([KT, NK, H], f16)
            w2_sb = wpool.tile([H, OUT], f16)
            bias_sb = wpool.tile([KT, 2], f32)
            wr_sb = wpool.tile([KT, H], f16)
            xr_sb = wpool.tile([KT, NQ, NB], f8)

            # The two HWDGE rings share one descriptor generator, so what
            # matters is the global command order and descriptor size:
            # 2-tile groups (6144B per-partition lines) sustain ~300+GB/s
            # combined, singles only ~150 (starves the PE at 296GB/s).
            # wm must lead a ring (the SWDGE queue only starts generating
            # at ~11.8us — far too late for the first chain); the first
            # x pair leads the other ring so wm+pair(0,1) land ~13.3us.
            nc.sync.dma_start(w_sb[:], wm[:])
            for gi, t in enumerate(range(0, NT, 2)):
                eng = (nc.scalar, nc.sync)[gi % 2]
                eng.dma_start(x_sb[:, t:t + 2, :, :], xtp[:, t:t + 2, :, :])
            # small tensors ride the SWDGE queue (its ~11.8us start is
            # fine: xr4/bias are first needed ~17us, w2t ~20us)
            nc.gpsimd.dma_start(bias_sb[:], biasd[:])
            nc.gpsimd.dma_start(wr_sb[:], wr4[:])
            nc.gpsimd.dma_start(xr_sb[:], xr4[:])
            nc.gpsimd.dma_start(w2_sb[:], w2t[:])

            warm_x = wpool.tile([KT, NB], f16)
            nc.vector.memset(warm_x[:], 0.0)
            warm_ps = ps1pool.tile([H, NB], f32, tag="ps1")
            for _ in range(N_WARM):
                nc.tensor.matmul(warm_ps[:], lhsT=warm_x[:, 0:H],
                                 rhs=warm_x[:], start=True, stop=True)

            def mm2_store_burst(q, hs):
                # 4 col-tiled mm2 passes into one shared PSUM bank
                ps2 = ps2pool.tile([KT, NB], f32, name="ps2")
                for j in range(4):
                    nc.tensor.matmul(
                        ps2[32 * j:32 * j + OUT, :],
                        lhsT=w2_sb[:], rhs=hs[j][:],
                        start=True, stop=True,
                        tile_position=(0, 32 * j),
                    )
                # b2-add in partition halves on both engines so each half
                # can store as soon as it lands
                o_sb = opool.tile([KT, NB], f16, name="o_sb")
                nc.vector.tensor_scalar_add(o_sb[0:64, :], ps2[0:64, :],
                                            bias_sb[0:64, 1:2])
                nc.scalar.activation(o_sb[64:KT, :], ps2[64:KT, :], ident,
                                     bias=bias_sb[64:KT, 1:2])
                if q == NQ - 1:
                    # final store on the (by now idle) HWDGE rings: each
                    # half fires the moment its bias-add completes
                    nc.sync.dma_start(out4[0:64, q, :], o_sb[0:64, :])
                    nc.scalar.dma_start(out4[64:KT, q, :], o_sb[64:KT, :])
                else:
                    nc.gpsimd.dma_start(out4[:, q, :], o_sb[:])

            prev = None
            for q in range(NQ):
                ps1s = []
                for j in range(4):
                    t = 4 * q + j
                    ps1 = ps1pool.tile([H, NB], f32, name="ps1")
                    for k in range(NK):
                        nc.tensor.matmul(
                            ps1[:],
                            lhsT=w_sb[:, k, :],
                            rhs=x_sb[:, t, k, :],
                            start=(k == 0),
                            stop=False,
                        )
                    ps1s.append(ps1)
                # remainder burst: 4 row-tiled K=16 passes, one per strip
                for j in range(4):
                    nc.tensor.matmul(
                        ps1s[j][:],
                        lhsT=wr_sb[32 * j:32 * j + DR, :],
                        rhs=xr_sb[32 * j:32 * j + DR, q, :],
                        start=False, stop=True,
                        tile_position=(32 * j, 0),
                    )
                # relu split across DVE and Act so the chain is 2 deep
                # per engine; emission order lets both start immediately
                hs = [None] * 4
                for j in (0, 2, 1, 3):
                    h_sb = hpool.tile([H, NB], f16, name="h_sb")
                    if j < 2:
                        nc.vector.tensor_scalar(
                            h_sb[:], ps1s[j][:], bias_sb[:, 0:1], 0.0,
                            mybir.AluOpType.add, mybir.AluOpType.max)
                    else:
                        nc.scalar.activation(h_sb[:], ps1s[j][:], relu,
                                             bias=bias_sb[:, 0:1])
                    hs[j] = h_sb
                if prev is not None:
                    mm2_store_burst(*prev)
                prev = (q, hs)
            mm2_store_burst(*prev)

    nc.compile()
    return nc


def _get_nc():
    if "nc" not in _CACHE:
        _CACHE["nc"] = _build_nc()
    return _CACHE["nc"]


def _fold_weights(conv_w: np.ndarray, w1: np.ndarray) -> np.ndarray:
    """W_eff[784, 128]: h_pre = x @ W_eff  ==  conv(x) @ w1.T  (float64 accum)."""
    w1k = w1.reshape(H, 26, 26).transpose(1, 2, 0).astype(np.float64)  # [i,j,k]
    cw = conv_w.astype(np.float64)
    W = np.zeros((28, 28, H), np.float64)
    for di in range(3):
        for dj in range(3):
            W[di:di + 26, dj:dj + 26, :] += cw[di, dj] * w1k
    return W.reshape(D, H).astype(np.float32)


def make_in_maps(x, conv_w, w1, b1, w2, b2):
    x = np.asarray(x, np.float32)
    weff = _fold_weights(np.asarray(conv_w, np.float32),
                         np.asarray(w1, np.float32))
    wm = np.ascontiguousarray(
        weff[:DM].reshape(NK, KT, H).transpose(1, 0, 2)).astype(np.float16)
    wr4 = np.zeros((KT, H), np.float16)
    for j in range(4):
        wr4[32 * j:32 * j + DR] = weff[DM:].astype(np.float16)
    w2t = np.ascontiguousarray(np.asarray(w2, np.float32).T).astype(np.float16)
    biasd = np.zeros((KT, 2), np.float32)
    biasd[:, 0] = np.asarray(b1, np.float32)
    for j in range(4):
        biasd[32 * j:32 * j + OUT, 1] = np.asarray(b2, np.float32)
    in_maps = []
    for i in range(N_CORES):
        xq = x[i * B_SH:(i + 1) * B_SH].astype(ml_dtypes.float8_e3m4)
        xtp = xq[:, :DM].reshape(NT, NB, NK, KT).transpose(3, 0, 2, 1)
        # [q, j, c, r] -> strip layout [4j, r, q, c] padded to 32 rows/strip
        r16 = xq[:, DM:].reshape(NQ, 4, NB, DR).transpose(1, 3, 0, 2)
        xr4 = np.zeros((4, 32, NQ, NB), ml_dtypes.float8_e3m4)
        xr4[:, :DR] = r16
        in_maps.append({"xtp": np.ascontiguousarray(xtp),
                        "xr4": np.ascontiguousarray(xr4.reshape(KT, NQ, NB)),
                        "wm": wm, "wr4": wr4, "w2t": w2t, "biasd": biasd})
    return in_maps


def kernel(x, conv_w, w1, b1, w2, b2):
    nc = _get_nc()
    in_maps = make_in_maps(x, conv_w, w1, b1, w2, b2)
    res = run_bass_kernel_spmd(nc, in_maps, list(range(N_CORES)))
    # out4[32j+r, q, c] -> out[(4q+j)*512+c, r]
    outs = []
    for i in range(N_CORES):
        o4 = res.results[i]["out4"].astype(np.float32)
        o4 = o4.reshape(4, 32, NQ, NB)[:, :OUT]
        outs.append(o4.transpose(1, 2, 0, 3).reshape(OUT, B_SH))
    out = np.concatenate(outs, axis=1)
    return np.ascontiguousarray(out.T)  # [65536, 10] float32
